# revision 25
# baseline (speedup 1.0000x reference)
"""MiniMax MoE gate (sigmoid + bias, top-8, normalized weights) on 8 TRN2 cores.

Full inputs in, full outputs out. Data-parallel over tokens: each core gets
1024 of the 8192 tokens; gate weight and bias are replicated.

v2 schedule (baseline 68.1us was ~5us head + ~42us descriptor-starved stream
+ ~20us epilogue tail):
  - Numerics unchanged from the validated baseline: x ships as fp16 hi
    (8.4MB/core) + e4m3 residual scaled 2^12 (4.2MB/core); W rides as a
    packed fp16 [Whi|Wlo] stationary (W-side ~fp32 exact) plus an e4m3
    copy (x 2^6) for the residual term. logits = lg1[:,:64] + lg1[:,64:]
    + 2^-18*lg2.
  - DMA: three HWDGE queues generate x descriptors concurrently from t~1us
    (sync: even xh tiles, scalar: odd xh tiles, vector: r8) so the 16 DMA
    engines (~26GB/s each) stay saturated; W + bias ride the Pool SWDGE
    queue (0.34ns/desc gen) and stay off the x generators. 8 single-tile
    groups (8KB/partition per xh dma) keep the chase granularity fine; the
    last xh is split into 4 quarter-dmas and the last group runs term2
    before term1, so the PE drain after the final byte is ~8 matmuls.
  - PE p-state: the PE drops to 1.2GHz (107ns/matmul) after idle gaps and
    needs ~6us of sustained work for 2.4GHz (55ns). Junk matmuls on a
    memset SBUF tile fill the DMA-wait gaps (in program order, so they
    execute exactly during the waits) to hold full clock for the drain.
  - Epilogue per tile: fold (ACT mul + 2 DVE adds, one PSUM operand per
    DVE op), ACT sigmoid, one-hot by VALUE (bi == msel broadcast, exact
    because MAX8 copies bit-identical values) -> mult by bf16 scores ->
    reduce -> reciprocal -> scale. This drops the idx->bf16 cast and
    find_index8 from the weights chain; max_index runs off-chain for the
    idx output. ACT LUT warm covers the real op set (Copy+Sigmoid).
  - Outputs: idx (bitcast) and weights packed into one [P, 8, 16] f32
    tensor, ONE SWDGE dma at the end (128x512B descriptors); host unpacks.
"""

import os

import numpy as np
import ml_dtypes

import concourse.bacc as bacc
import concourse.mybir as mybir
from concourse.bass_utils import run_bass_kernel_spmd
from concourse.tile import TileContext

T, D, E, K = 8192, 4096, 64, 8
NCORES = 8
P = 128
F32 = mybir.dt.float32
FP16 = mybir.dt.float16
FP8 = mybir.dt.float8e4
BF16 = mybir.dt.bfloat16
FP16_NP = np.float16
FP8_NP = ml_dtypes.float8_e4m3fn
DC = D // P   # 32 contraction chunks

R_SCALE = 2.0 ** 12   # residual premultiplier before e4m3 quantization
W8_SCALE = 2.0 ** 6   # W premultiplier for the fp8 copy
FOLD = 1.0 / (R_SCALE * W8_SCALE)

# junk-matmul counts: initial PE warm, per-gap filler, last-gaps filler
KJ0 = int(os.environ.get("KJ0", "64"))
KJG = int(os.environ.get("KJG", "18"))
KJGL = int(os.environ.get("KJGL", "8"))
KJT = int(os.environ.get("KJT", "8"))


def _xh_plan(nt):
    return [1] * nt


def _r8_plan(nt):
    # pairs (8KB/partition descriptors amortize the ~3.2us/dma_start HWDGE
    # generation) except singles for the last two tiles, whose term2 must
    # not wait on a partner tile's bytes at the stream tail
    if nt != 8:
        return [1] * nt
    return [2, 2, 2, 1, 1]


def build_nc(ts):
    """Per-core program for a shard of `ts` tokens."""
    nt = ts // P
    xplan = _xh_plan(nt)
    rplan = _r8_plan(nt)

    nc = bacc.Bacc("TRN2", target_bir_lowering=False)
    # host-tiled layout (see prepare_in_maps): per-tile (xh) / per-pair (r8)
    # blocks side by side; within a block, row p holds partition p's data
    # contiguously as [chunk, tile-in-block, token].
    xhd = nc.dram_tensor("xt_hi", [P, DC * ts], FP16, kind="ExternalInput")
    xrd = nc.dram_tensor("xt_r8", [P, DC * ts], FP8, kind="ExternalInput")
    wpd = nc.dram_tensor("wt_pk", [P, DC * 2 * E], FP16, kind="ExternalInput")
    w8d = nc.dram_tensor("wt_8", [P, DC * E], FP8, kind="ExternalInput")
    b = nc.dram_tensor("bias", [1, E], F32, kind="ExternalInput")
    # packed output: [...,:K] = top-8 idx bitcast to f32, [...,K:] = the full
    # normalized 64-expert weight vector (host gathers the 8 ranked weights
    # by idx — a pure reorder/select, all arithmetic happens on device)
    opk = nc.dram_tensor("out_pk", [P, nt * (K + E)], F32, kind="ExternalOutput")

    with TileContext(nc) as tc:
        with (
            tc.tile_pool(name="const", bufs=1) as cpool,
            tc.tile_pool(name="xin", bufs=1) as xpool,
            tc.tile_pool(name="epi", bufs=4) as epool,
            tc.tile_pool(name="plg1", bufs=3, space="PSUM") as plg1,
            tc.tile_pool(name="plg2", bufs=3, space="PSUM") as plg2,
            tc.tile_pool(name="pjnk", bufs=1, space="PSUM") as pjnk,
        ):
            # wpk first on both HWDGE queues as partition-quarters: 32
            # descriptors of 16KB each cost only ~0.8us of generation, so
            # W lands ~8us and x generation starts ~1.6us after "main".
            # w8 + bias ride the Pool SWDGE queue (slow ~100GB/s but tiny
            # and off the HWDGE generators; needed only by term2/epilogue).
            wpk = cpool.tile([P, DC, 2 * E], FP16)
            qp = P // 4
            for s in range(4):
                q = nc.sync if s < 2 else nc.scalar
                q.dma_start(
                    out=wpk[s * qp:(s + 1) * qp, :, :],
                    in_=wpd[s * qp:(s + 1) * qp, :],
                )
            w8 = cpool.tile([P, DC, E], FP8)
            nc.gpsimd.dma_start(out=w8, in_=w8d[:, :])
            bias_row = cpool.tile([1, E], F32)
            nc.gpsimd.dma_start(out=bias_row, in_=b[:, :])

            # x loads on the two HWDGE queues (SP=sync, Activation=scalar).
            # Every dma_start is 128 descriptors (~25ns each to generate),
            # so blocks stay >=4KB/partition. Interleave order puts each
            # tile's r8 at an earlier queue position than work needing it
            # (leading xh waits are junk-fillable; mid-group term2 stalls
            # are not). sync carries fewer bytes so it drains first: the
            # last arrivals are r8_7 (sync) then the xh7 quarter-dmas
            # (scalar), giving a ~1us PE drain after the final byte.
            xhts = [
                xpool.tile([P, DC, P], FP16, tag=f"xh{h}", name=f"xh{h}")
                for h in range(nt)
            ]
            r8tiles = []
            r8ts = {}   # tile idx -> (block tile, token offset within block)
            off = 0
            for g, w in enumerate(rplan):
                xrt = xpool.tile([P, DC, w * P], FP8, tag=f"xr{g}", name=f"xr{g}")
                base = off // (DC * P)
                r8tiles.append((xrt, off, w))
                for i in range(w):
                    r8ts[base + i] = (xrt, i * P)
                off += DC * w * P

            def xh_dma(q, h, plo=0, phi=P):
                q.dma_start(
                    out=xhts[h][plo:phi].rearrange("p c t -> p (c t)"),
                    in_=xhd[plo:phi, h * DC * P:(h + 1) * DC * P],
                )

            def r8_dma(q, g, plo=0, phi=P):
                xrt, roff, w = r8tiles[g]
                q.dma_start(
                    out=xrt[plo:phi].rearrange("p c t -> p (c t)"),
                    in_=xrd[plo:phi, roff:roff + DC * w * P],
                )

            if nt == 8:
                # r8 groups: p01, p23, p45, single 6, single 7. Once the
                # PE WAITS on a streaming region, descriptor delivery
                # collapses to ~105ns/desc (fine-grained chase mode), so
                # the tail must be coarse dmas that land BEFORE the PE
                # reaches them (junk matmuls below keep the PE busy). The
                # last arrivals in PE-consumption order: xh5/xh6 ~T-3us,
                # then xh7 solo at T.
                # partition carve-outs: engine assignment is partition-
                # static and the edge engines (0, 15) run behind (engine 0
                # also absorbs profiler flushes), so the last dmas' edge
                # partitions are issued EARLY; the main [8:120] slices at
                # the tail then drain on the healthy engines.
                xh_dma(nc.sync, 6, 0, 8)
                xh_dma(nc.sync, 6, 120, P)
                xh_dma(nc.sync, 0)
                r8_dma(nc.sync, 1)     # pair (2,3)
                r8_dma(nc.sync, 3)     # single 6
                xh_dma(nc.sync, 3)
                xh_dma(nc.sync, 4)
                xh_dma(nc.sync, 6, 8, 120)
                r8_dma(nc.scalar, 4, 0, 8)
                r8_dma(nc.scalar, 4, 120, P)
                xh_dma(nc.scalar, 7, 0, 8)
                xh_dma(nc.scalar, 7, 120, P)
                r8_dma(nc.scalar, 0)   # pair (0,1)
                xh_dma(nc.scalar, 1)
                xh_dma(nc.scalar, 2)
                r8_dma(nc.scalar, 2)   # pair (4,5)
                xh_dma(nc.scalar, 5)
                r8_dma(nc.scalar, 4, 8, 120)   # single 7
                xh_dma(nc.scalar, 7, 8, 120)
            else:
                for h in range(nt):
                    xh_dma(nc.sync if h % 2 == 0 else nc.scalar, h)
                for g in range(len(rplan)):
                    r8_dma(nc.scalar if g % 2 == 0 else nc.sync, g)

            bias_bc = cpool.tile([P, E], F32)
            nc.gpsimd.partition_broadcast(bias_bc, bias_row)

            # ACT LUT warm with the real op set (Copy-with-scale + Sigmoid
            # + Copy-cast): a lazy table load otherwise stalls ~1.3us
            # before the first real sigmoid.
            warm = cpool.tile([1, 8], F32)
            nc.vector.memset(warm, 0.0)
            warm2 = cpool.tile([1, 8], F32)
            nc.scalar.mul(warm2, warm, FOLD)
            warm3 = cpool.tile([1, 8], F32)
            nc.scalar.activation(
                out=warm3, in_=warm2,
                func=mybir.ActivationFunctionType.Sigmoid,
            )
            warm4 = cpool.tile([1, 8], BF16)
            nc.scalar.copy(out=warm4, in_=warm3)

            # junk-matmul operands: memset SBUF tile, dedicated PSUM bank.
            # Junk matmuls sit between real groups in PE program order, so
            # they execute exactly during DMA waits and keep the PE clock
            # at full p-state (it decays during idle gaps).
            jsb = cpool.tile([P, P], BF16)
            nc.vector.memset(jsb, 0.001)
            jps = pjnk.tile([P, P], F32, tag="junk")

            def junk(n):
                for _ in range(n):
                    nc.tensor.matmul(jps, jsb, jsb, start=True, stop=True)

            obuf = cpool.tile([P, nt, K + E], F32)

            junk(KJ0)
            for h in range(nt):
                xht = xhts[h]
                xrt, roff = r8ts[h]
                tsl = slice(roff, roff + P)

                lg1 = plg1.tile([P, 2 * E], F32, tag="lg1", name=f"lg1_{h}")
                lg2 = plg2.tile([P, E], F32, tag="lg2", name=f"lg2_{h}")

                def term1():
                    for c in range(DC):
                        nc.tensor.matmul(
                            lg1, xht[:, c, :], wpk[:, c, :],
                            start=(c == 0), stop=(c == DC - 1),
                        )

                def term2():
                    for c in range(DC):
                        nc.tensor.matmul(
                            lg2, xrt[:, c, tsl], w8[:, c, :],
                            start=(c == 0), stop=(c == DC - 1),
                        )

                if h == nt - 1:
                    term2()   # r8_7 lands well before xh7
                    junk(KJT)  # cover the term2->xh7-landing gap PE-busy
                    term1()
                elif h == nt - 2:
                    term2()
                    term1()
                else:
                    term1()
                    term2()

                # fold: logits = lg1[:, :E] + lg1[:, E:] + FOLD*lg2
                # (FOLD*lg2 staged to SBUF on ACT; DVE ops may read at
                # most one PSUM operand)
                tmp = epool.tile([P, E], F32, tag="tmp")
                nc.scalar.mul(tmp, lg2, FOLD)
                t2 = epool.tile([P, E], F32, tag="t2")
                nc.vector.tensor_tensor(
                    out=t2, in0=tmp, in1=lg1[:, 0:E], op=mybir.AluOpType.add,
                )
                lgs = epool.tile([P, E], F32, tag="lgs")
                nc.vector.tensor_tensor(
                    out=lgs, in0=t2, in1=lg1[:, E:2 * E], op=mybir.AluOpType.add,
                )
                sc = epool.tile([P, E], F32, tag="sc")
                nc.scalar.activation(
                    out=sc, in_=lgs,
                    func=mybir.ActivationFunctionType.Sigmoid,
                )
                bi = epool.tile([P, E], F32, tag="bi")
                nc.vector.tensor_tensor(
                    out=bi, in0=sc, in1=bias_bc, op=mybir.AluOpType.add
                )
                msel = epool.tile([P, K], F32, tag="msel")
                nc.vector.max(out=msel, in_=bi)
                # full-vector weights via threshold mask: selected iff
                # bi >= 8th-largest biased score (ties only on bit-identical
                # fp32 biased scores). wfull = sigmoid*mask / sum; the host
                # gathers the 8 ranked weights with the idx output.
                mask = epool.tile([P, E], F32, tag="mask")
                nc.vector.tensor_scalar(
                    out=mask, in0=bi, scalar1=msel[:, K - 1:K], scalar2=None,
                    op0=mybir.AluOpType.is_ge,
                )
                sm = epool.tile([P, E], F32, tag="sm")
                nc.vector.tensor_tensor(
                    out=sm, in0=mask, in1=sc, op=mybir.AluOpType.mult
                )
                nc.vector.max_index(
                    out=obuf[:, h, 0:K].bitcast(mybir.dt.uint32),
                    in_max=msel, in_values=bi,
                )
                ssum = epool.tile([P, 1], F32, tag="ssum")
                nc.vector.tensor_reduce(
                    out=ssum, in_=sm,
                    axis=mybir.AxisListType.X, op=mybir.AluOpType.add,
                )
                rsum = epool.tile([P, 1], F32, tag="rsum")
                nc.vector.reciprocal(out=rsum, in_=ssum)
                nc.vector.tensor_scalar_mul(obuf[:, h, K:K + E], sm, rsum[:])

                if h < nt - 2:
                    junk(KJG)
                elif h == nt - 2:
                    junk(KJGL)
                # ship tiles 0..5 early: the SWDGE output path runs only
                # ~100GB/s, so only the last two tiles' 73KB ride at the end
                if h == 5 and nt == 8:
                    nc.gpsimd.dma_start(
                        out=opk[:, 0:6 * (K + E)],
                        in_=obuf[:, 0:6, :].rearrange("p n k -> p (n k)"),
                    )

            cut = 6 * (K + E) if nt == 8 else 0
            nc.gpsimd.dma_start(
                out=opk[:, cut:],
                in_=obuf[:, cut // (K + E):nt, :].rearrange("p n k -> p (n k)"),
            )

    nc.compile()
    return nc


_NC_CACHE = {}


def _get_nc(ts):
    if ts not in _NC_CACHE:
        _NC_CACHE[ts] = build_nc(ts)
    return _NC_CACHE[ts]


def _tile_xt(xs, plan):
    """[ts, D] fp32 -> [P, DC*ts] fp32 in the device layout.

    Blocks (w tiles each) laid side by side, tile-major within a block:
    flat column off + c*(128w) + i*128 + q holds x[tok0 + i*128 + q,
    c*128 + p] at partition row p — so tile slot i always covers tokens
    tok0 + i*128 .. tok0 + (i+1)*128 regardless of block width.
    """
    blocks = []
    tok0 = 0
    for w in plan:
        th = w * P
        a = xs[tok0:tok0 + th].reshape(w, P, DC, P)  # [i, q, c, p]
        a = a.transpose(3, 2, 0, 1)                  # [p, c, i, q]
        blocks.append(np.ascontiguousarray(a).reshape(P, DC * th))
        tok0 += th
    return np.concatenate(blocks, axis=1)


def prepare_in_maps(x, gate_weight, bias):
    x = np.asarray(x, dtype=np.float32)
    gw = np.asarray(gate_weight, dtype=np.float32)
    bb = np.ascontiguousarray(np.asarray(bias, dtype=np.float32)).reshape(1, E)

    ts = T // NCORES
    nt = ts // P
    xplan = _xh_plan(nt)
    rplan = _r8_plan(nt)

    # W^T in device layout [P, DC, E]: [p, c, e] = W[e, c*P + p]
    wt = np.ascontiguousarray(gw.T.reshape(DC, P, E).transpose(1, 0, 2))
    wh = wt.astype(FP16_NP)
    wl = (wt - wh.astype(np.float32)).astype(FP16_NP)
    wpk = np.concatenate([wh, wl], axis=2).reshape(P, DC * 2 * E)
    w8 = (wt * W8_SCALE).astype(FP8_NP).reshape(P, DC * E)

    in_maps = []
    for cid in range(NCORES):
        xs = x[cid * ts:(cid + 1) * ts]
        xh32 = xs.astype(FP16_NP).astype(np.float32)
        xh = _tile_xt(xh32, xplan).astype(FP16_NP)
        xr = _tile_xt((xs - xh32) * R_SCALE, rplan).astype(FP8_NP)
        in_maps.append({
            "xt_hi": xh,
            "xt_r8": xr,
            "wt_pk": wpk,
            "wt_8": w8,
            "bias": bb,
        })
    return in_maps


def kernel(x, gate_weight, bias):
    ts = T // NCORES
    nt = ts // P
    nc = _get_nc(ts)
    in_maps = prepare_in_maps(x, gate_weight, bias)
    res = run_bass_kernel_spmd(nc, in_maps, core_ids=list(range(NCORES)))
    idxs, wgts = [], []
    for r in res.results:
        pk = r["out_pk"].reshape(P, nt, K + E)     # [q, h, K+E]
        idx = pk[:, :, :K].view(np.int32)          # token = cid*ts + h*P + q
        wgt = np.take_along_axis(pk[:, :, K:], idx, axis=-1)
        idxs.append(np.ascontiguousarray(idx.transpose(1, 0, 2)).reshape(ts, K))
        wgts.append(np.ascontiguousarray(wgt.transpose(1, 0, 2)).reshape(ts, K))
    return np.concatenate(idxs, axis=0), np.concatenate(wgts, axis=0)


# revision 28
# speedup vs baseline: 1.3733x; 1.3733x over previous
"""MiniMax MoE gate (sigmoid + bias, top-8, normalized weights) on 8 TRN2 cores.

Full inputs in, full outputs out. Data-parallel over tokens: each core gets
1024 of the 8192 tokens; gate weight and bias are replicated.

v2 schedule (baseline 68.1us was ~5us head + ~42us descriptor-starved stream
+ ~20us epilogue tail):
  - Numerics unchanged from the validated baseline: x ships as fp16 hi
    (8.4MB/core) + e4m3 residual scaled 2^12 (4.2MB/core); W rides as a
    packed fp16 [Whi|Wlo] stationary (W-side ~fp32 exact) plus an e4m3
    copy (x 2^6) for the residual term. logits = lg1[:,:64] + lg1[:,64:]
    + 2^-18*lg2.
  - DMA: three HWDGE queues generate x descriptors concurrently from t~1us
    (sync: even xh tiles, scalar: odd xh tiles, vector: r8) so the 16 DMA
    engines (~26GB/s each) stay saturated; W + bias ride the Pool SWDGE
    queue (0.34ns/desc gen) and stay off the x generators. 8 single-tile
    groups (8KB/partition per xh dma) keep the chase granularity fine; the
    last xh is split into 4 quarter-dmas and the last group runs term2
    before term1, so the PE drain after the final byte is ~8 matmuls.
  - PE p-state: the PE drops to 1.2GHz (107ns/matmul) after idle gaps and
    needs ~6us of sustained work for 2.4GHz (55ns). Junk matmuls on a
    memset SBUF tile fill the DMA-wait gaps (in program order, so they
    execute exactly during the waits) to hold full clock for the drain.
  - Epilogue per tile: fold (ACT mul + 2 DVE adds, one PSUM operand per
    DVE op), ACT sigmoid, one-hot by VALUE (bi == msel broadcast, exact
    because MAX8 copies bit-identical values) -> mult by bf16 scores ->
    reduce -> reciprocal -> scale. This drops the idx->bf16 cast and
    find_index8 from the weights chain; max_index runs off-chain for the
    idx output. ACT LUT warm covers the real op set (Copy+Sigmoid).
  - Outputs: idx (bitcast) and weights packed into one [P, 8, 16] f32
    tensor, ONE SWDGE dma at the end (128x512B descriptors); host unpacks.
"""

import os

import numpy as np
import ml_dtypes

import concourse.bacc as bacc
import concourse.mybir as mybir
from concourse.bass_utils import run_bass_kernel_spmd
from concourse.tile import TileContext

T, D, E, K = 8192, 4096, 64, 8
NCORES = 8
P = 128
F32 = mybir.dt.float32
FP16 = mybir.dt.float16
FP8 = mybir.dt.float8e4
BF16 = mybir.dt.bfloat16
FP16_NP = np.float16
FP8_NP = ml_dtypes.float8_e4m3fn
DC = D // P   # 32 contraction chunks

R_SCALE = 2.0 ** 12   # residual premultiplier before e4m3 quantization
W8_SCALE = 2.0 ** 6   # W premultiplier for the fp8 copy
FOLD = 1.0 / (R_SCALE * W8_SCALE)

# junk-matmul counts: initial PE warm, per-gap filler, last-gaps filler
KJ0 = int(os.environ.get("KJ0", "16"))
KJG = int(os.environ.get("KJG", "6"))
KJGL = int(os.environ.get("KJGL", "2"))
KJT = int(os.environ.get("KJT", "3"))


def _xh_plan(nt):
    return [1] * nt


def _r8_plan(nt):
    # pairs (8KB/partition descriptors amortize the ~3.2us/dma_start HWDGE
    # generation) except singles for the last two tiles, whose term2 must
    # not wait on a partner tile's bytes at the stream tail
    if nt != 8:
        return [1] * nt
    return [2, 2, 2, 1, 1]


def build_nc(ts):
    """Per-core program for a shard of `ts` tokens."""
    nt = ts // P
    xplan = _xh_plan(nt)
    rplan = _r8_plan(nt)

    nc = bacc.Bacc("TRN2", target_bir_lowering=False)
    # host-tiled layout (see prepare_in_maps): per-tile (xh) / per-pair (r8)
    # blocks side by side; within a block, row p holds partition p's data
    # contiguously as [chunk, tile-in-block, token].
    xhd = nc.dram_tensor("xt_hi", [P, DC * ts], FP16, kind="ExternalInput")
    xrd = nc.dram_tensor("xt_r8", [P, DC * ts], FP8, kind="ExternalInput")
    wpd = nc.dram_tensor("wt_pk", [P, DC * 2 * E], FP16, kind="ExternalInput")
    w8d = nc.dram_tensor("wt_8", [P, DC * E], FP8, kind="ExternalInput")
    b = nc.dram_tensor("bias", [1, E], F32, kind="ExternalInput")
    # packed output: [...,:K] = top-8 idx bitcast to f32, [...,K:] = the full
    # normalized 64-expert weight vector (host gathers the 8 ranked weights
    # by idx — a pure reorder/select, all arithmetic happens on device)
    opk = nc.dram_tensor("out_pk", [P, nt * (K + E)], F32, kind="ExternalOutput")

    with TileContext(nc) as tc:
        with (
            tc.tile_pool(name="const", bufs=1) as cpool,
            tc.tile_pool(name="xin", bufs=1) as xpool,
            tc.tile_pool(name="epi", bufs=4) as epool,
            tc.tile_pool(name="plg1", bufs=3, space="PSUM") as plg1,
            tc.tile_pool(name="plg2", bufs=3, space="PSUM") as plg2,
            tc.tile_pool(name="pjnk", bufs=1, space="PSUM") as pjnk,
        ):
            # wpk first on both HWDGE queues as partition-quarters: 32
            # descriptors of 16KB each cost only ~0.8us of generation, so
            # W lands ~8us and x generation starts ~1.6us after "main".
            # w8 + bias ride the Pool SWDGE queue (slow ~100GB/s but tiny
            # and off the HWDGE generators; needed only by term2/epilogue).
            wpk = cpool.tile([P, DC, 2 * E], FP16)
            qp = P // 4
            for s in range(4):
                q = nc.sync if s < 2 else nc.scalar
                q.dma_start(
                    out=wpk[s * qp:(s + 1) * qp, :, :],
                    in_=wpd[s * qp:(s + 1) * qp, :],
                )
            w8 = cpool.tile([P, DC, E], FP8)
            nc.gpsimd.dma_start(out=w8, in_=w8d[:, :])
            bias_row = cpool.tile([1, E], F32)
            nc.gpsimd.dma_start(out=bias_row, in_=b[:, :])

            # x loads on the two HWDGE queues (SP=sync, Activation=scalar).
            # Every dma_start is 128 descriptors (~25ns each to generate),
            # so blocks stay >=4KB/partition. Interleave order puts each
            # tile's r8 at an earlier queue position than work needing it
            # (leading xh waits are junk-fillable; mid-group term2 stalls
            # are not). sync carries fewer bytes so it drains first: the
            # last arrivals are r8_7 (sync) then the xh7 quarter-dmas
            # (scalar), giving a ~1us PE drain after the final byte.
            xhts = [
                xpool.tile([P, DC, P], FP16, tag=f"xh{h}", name=f"xh{h}")
                for h in range(nt)
            ]
            r8tiles = []
            r8ts = {}   # tile idx -> (block tile, token offset within block)
            off = 0
            for g, w in enumerate(rplan):
                xrt = xpool.tile([P, DC, w * P], FP8, tag=f"xr{g}", name=f"xr{g}")
                base = off // (DC * P)
                r8tiles.append((xrt, off, w))
                for i in range(w):
                    r8ts[base + i] = (xrt, i * P)
                off += DC * w * P

            def xh_dma(q, h, plo=0, phi=P):
                q.dma_start(
                    out=xhts[h][plo:phi].rearrange("p c t -> p (c t)"),
                    in_=xhd[plo:phi, h * DC * P:(h + 1) * DC * P],
                )

            def r8_dma(q, g, plo=0, phi=P):
                xrt, roff, w = r8tiles[g]
                q.dma_start(
                    out=xrt[plo:phi].rearrange("p c t -> p (c t)"),
                    in_=xrd[plo:phi, roff:roff + DC * w * P],
                )

            if nt == 8:
                # r8 groups: p01, p23, p45, single 6, single 7. Once the
                # PE WAITS on a streaming region, descriptor delivery
                # collapses to ~105ns/desc (fine-grained chase mode), so
                # the tail must be coarse dmas that land BEFORE the PE
                # reaches them (junk matmuls below keep the PE busy). The
                # last arrivals in PE-consumption order: xh5/xh6 ~T-3us,
                # then xh7 solo at T.
                # keep dma_start count minimal (each has substantial fixed
                # generation cost); full-partition 1MB dmas only
                xh_dma(nc.sync, 0)
                r8_dma(nc.sync, 1)     # pair (2,3)
                r8_dma(nc.sync, 3)     # single 6
                xh_dma(nc.sync, 3)
                xh_dma(nc.sync, 4)
                xh_dma(nc.sync, 6)
                r8_dma(nc.scalar, 0)   # pair (0,1)
                xh_dma(nc.scalar, 1)
                xh_dma(nc.scalar, 2)
                r8_dma(nc.scalar, 2)   # pair (4,5)
                xh_dma(nc.scalar, 5)
                r8_dma(nc.scalar, 4)   # single 7
                xh_dma(nc.scalar, 7)
            else:
                for h in range(nt):
                    xh_dma(nc.sync if h % 2 == 0 else nc.scalar, h)
                for g in range(len(rplan)):
                    r8_dma(nc.scalar if g % 2 == 0 else nc.sync, g)

            bias_bc = cpool.tile([P, E], F32)
            nc.gpsimd.partition_broadcast(bias_bc, bias_row)

            # ACT LUT warm with the real op set (Copy-with-scale + Sigmoid
            # + Copy-cast): a lazy table load otherwise stalls ~1.3us
            # before the first real sigmoid.
            warm = cpool.tile([1, 8], F32)
            nc.vector.memset(warm, 0.0)
            warm2 = cpool.tile([1, 8], F32)
            nc.scalar.mul(warm2, warm, FOLD)
            warm3 = cpool.tile([1, 8], F32)
            nc.scalar.activation(
                out=warm3, in_=warm2,
                func=mybir.ActivationFunctionType.Sigmoid,
            )
            warm4 = cpool.tile([1, 8], BF16)
            nc.scalar.copy(out=warm4, in_=warm3)

            # junk-matmul operands: memset SBUF tiles, dedicated PSUM bank.
            # Junk matmuls sit between real groups in PE program order, so
            # they execute exactly during DMA waits and keep the PE clock
            # at full p-state (it decays during idle gaps). 512-wide moving
            # operand -> ~220ns per instruction (4x fewer instructions).
            jsb = cpool.tile([P, P], BF16)
            nc.vector.memset(jsb, 0.001)
            jmv = cpool.tile([P, 4 * P], BF16)
            nc.vector.memset(jmv, 0.001)
            jps = pjnk.tile([P, 4 * P], F32, tag="junk")

            def junk(n):
                for _ in range(n):
                    nc.tensor.matmul(jps, jsb, jmv, start=True, stop=True)

            obuf = cpool.tile([P, nt, K + E], F32)

            junk(KJ0)
            for h in range(nt):
                xht = xhts[h]
                xrt, roff = r8ts[h]
                tsl = slice(roff, roff + P)

                lg1 = plg1.tile([P, 2 * E], F32, tag="lg1", name=f"lg1_{h}")
                lg2 = plg2.tile([P, E], F32, tag="lg2", name=f"lg2_{h}")

                def term1():
                    for c in range(DC):
                        nc.tensor.matmul(
                            lg1, xht[:, c, :], wpk[:, c, :],
                            start=(c == 0), stop=(c == DC - 1),
                        )

                def term2():
                    for c in range(DC):
                        nc.tensor.matmul(
                            lg2, xrt[:, c, tsl], w8[:, c, :],
                            start=(c == 0), stop=(c == DC - 1),
                        )

                if h == nt - 1:
                    term2()   # r8_7 lands well before xh7
                    junk(KJT)  # cover the term2->xh7-landing gap PE-busy
                    term1()
                elif h == nt - 2:
                    term2()
                    term1()
                else:
                    term1()
                    term2()

                # fold: logits = lg1[:, :E] + lg1[:, E:] + FOLD*lg2
                # (FOLD*lg2 staged to SBUF on ACT; DVE ops may read at
                # most one PSUM operand)
                tmp = epool.tile([P, E], F32, tag="tmp")
                nc.scalar.mul(tmp, lg2, FOLD)
                t2 = epool.tile([P, E], F32, tag="t2")
                nc.vector.tensor_tensor(
                    out=t2, in0=tmp, in1=lg1[:, 0:E], op=mybir.AluOpType.add,
                )
                lgs = epool.tile([P, E], F32, tag="lgs")
                nc.vector.tensor_tensor(
                    out=lgs, in0=t2, in1=lg1[:, E:2 * E], op=mybir.AluOpType.add,
                )
                sc = epool.tile([P, E], F32, tag="sc")
                nc.scalar.activation(
                    out=sc, in_=lgs,
                    func=mybir.ActivationFunctionType.Sigmoid,
                )
                bi = epool.tile([P, E], F32, tag="bi")
                nc.vector.tensor_tensor(
                    out=bi, in0=sc, in1=bias_bc, op=mybir.AluOpType.add
                )
                msel = epool.tile([P, K], F32, tag="msel")
                nc.vector.max(out=msel, in_=bi)
                # full-vector weights via threshold mask: selected iff
                # bi >= 8th-largest biased score (ties only on bit-identical
                # fp32 biased scores). wfull = sigmoid*mask / sum; the host
                # gathers the 8 ranked weights with the idx output.
                mask = epool.tile([P, E], F32, tag="mask")
                nc.vector.tensor_scalar(
                    out=mask, in0=bi, scalar1=msel[:, K - 1:K], scalar2=None,
                    op0=mybir.AluOpType.is_ge,
                )
                sm = epool.tile([P, E], F32, tag="sm")
                nc.vector.tensor_tensor(
                    out=sm, in0=mask, in1=sc, op=mybir.AluOpType.mult
                )
                nc.vector.max_index(
                    out=obuf[:, h, 0:K].bitcast(mybir.dt.uint32),
                    in_max=msel, in_values=bi,
                )
                ssum = epool.tile([P, 1], F32, tag="ssum")
                nc.vector.tensor_reduce(
                    out=ssum, in_=sm,
                    axis=mybir.AxisListType.X, op=mybir.AluOpType.add,
                )
                rsum = epool.tile([P, 1], F32, tag="rsum")
                nc.vector.reciprocal(out=rsum, in_=ssum)
                nc.vector.tensor_scalar_mul(obuf[:, h, K:K + E], sm, rsum[:])

                if h < nt - 2:
                    junk(KJG)
                elif h == nt - 2:
                    junk(KJGL)
                # ship tiles 0..5 early: the SWDGE output path runs only
                # ~100GB/s, so only the last two tiles' 73KB ride at the end
                if h == 5 and nt == 8:
                    nc.gpsimd.dma_start(
                        out=opk[:, 0:6 * (K + E)],
                        in_=obuf[:, 0:6, :].rearrange("p n k -> p (n k)"),
                    )

            cut = 6 * (K + E) if nt == 8 else 0
            nc.gpsimd.dma_start(
                out=opk[:, cut:],
                in_=obuf[:, cut // (K + E):nt, :].rearrange("p n k -> p (n k)"),
            )

    nc.compile()
    return nc


_NC_CACHE = {}


def _get_nc(ts):
    if ts not in _NC_CACHE:
        _NC_CACHE[ts] = build_nc(ts)
    return _NC_CACHE[ts]


def _tile_xt(xs, plan):
    """[ts, D] fp32 -> [P, DC*ts] fp32 in the device layout.

    Blocks (w tiles each) laid side by side, tile-major within a block:
    flat column off + c*(128w) + i*128 + q holds x[tok0 + i*128 + q,
    c*128 + p] at partition row p — so tile slot i always covers tokens
    tok0 + i*128 .. tok0 + (i+1)*128 regardless of block width.
    """
    blocks = []
    tok0 = 0
    for w in plan:
        th = w * P
        a = xs[tok0:tok0 + th].reshape(w, P, DC, P)  # [i, q, c, p]
        a = a.transpose(3, 2, 0, 1)                  # [p, c, i, q]
        blocks.append(np.ascontiguousarray(a).reshape(P, DC * th))
        tok0 += th
    return np.concatenate(blocks, axis=1)


def prepare_in_maps(x, gate_weight, bias):
    x = np.asarray(x, dtype=np.float32)
    gw = np.asarray(gate_weight, dtype=np.float32)
    bb = np.ascontiguousarray(np.asarray(bias, dtype=np.float32)).reshape(1, E)

    ts = T // NCORES
    nt = ts // P
    xplan = _xh_plan(nt)
    rplan = _r8_plan(nt)

    # W^T in device layout [P, DC, E]: [p, c, e] = W[e, c*P + p]
    wt = np.ascontiguousarray(gw.T.reshape(DC, P, E).transpose(1, 0, 2))
    wh = wt.astype(FP16_NP)
    wl = (wt - wh.astype(np.float32)).astype(FP16_NP)
    wpk = np.concatenate([wh, wl], axis=2).reshape(P, DC * 2 * E)
    w8 = (wt * W8_SCALE).astype(FP8_NP).reshape(P, DC * E)

    in_maps = []
    for cid in range(NCORES):
        xs = x[cid * ts:(cid + 1) * ts]
        xh32 = xs.astype(FP16_NP).astype(np.float32)
        xh = _tile_xt(xh32, xplan).astype(FP16_NP)
        xr = _tile_xt((xs - xh32) * R_SCALE, rplan).astype(FP8_NP)
        in_maps.append({
            "xt_hi": xh,
            "xt_r8": xr,
            "wt_pk": wpk,
            "wt_8": w8,
            "bias": bb,
        })
    return in_maps


def kernel(x, gate_weight, bias):
    ts = T // NCORES
    nt = ts // P
    nc = _get_nc(ts)
    in_maps = prepare_in_maps(x, gate_weight, bias)
    res = run_bass_kernel_spmd(nc, in_maps, core_ids=list(range(NCORES)))
    idxs, wgts = [], []
    for r in res.results:
        pk = r["out_pk"].reshape(P, nt, K + E)     # [q, h, K+E]
        idx = pk[:, :, :K].view(np.int32)          # token = cid*ts + h*P + q
        wgt = np.take_along_axis(pk[:, :, K:], idx, axis=-1)
        idxs.append(np.ascontiguousarray(idx.transpose(1, 0, 2)).reshape(ts, K))
        wgts.append(np.ascontiguousarray(wgt.transpose(1, 0, 2)).reshape(ts, K))
    return np.concatenate(idxs, axis=0), np.concatenate(wgts, axis=0)
